# revision 37
# baseline (speedup 1.0000x reference)
"""Causal multi-head attention block (B=4, T=2048, C=1024, H=16, D=64) on 8 trn2 cores.

Sharding: core c -> (batch b = c//2, head-group g = c%2 covering heads 8g..8g+8).
Each core computes qkv projection for its batch restricted to its 8 heads,
flash-style causal attention in transposed orientation, and a partial output
projection written to DRAM in fp16; the host sums the two head-group partials
per batch during unshard (no device collective).

The attention inner loop is exp-throughput-bound on the scalar engine, so the
qkv projection is software-pipelined into it: only the first 512-column q/k
pass (kc-outer over 8 PSUM accumulators, started as soon as the first x/w DMA
chunks land) and the first four v tiles run up front; the remaining q/k
chunks, v tiles, and all output-projection jobs are drip-fed into the
ACT-gated tensor-engine idle slots between attention pairs, all sharing one
2-deep PSUM "job" ring. Causal masking is a single gpsimd affine_select on
the probability tile; diagonal tiles only compute the causally-live query
columns.

No transposes are needed on-device: q/k are produced in (d, t) layout (which
both the score matmul and its row-packed pairing want), v in (t, d) layout
(what PV wants), and the attention output appears directly in the (c_local, t)
layout that the output projection consumes as its stationary operand. Softmax
denominators come from a ones-column appended to v (M=65 PV matmuls).
"""
import sys

sys.path.insert(0, '/opt/trn_rl_repo')

from contextlib import ExitStack

import numpy as np

import concourse.bass as bass
import concourse.mybir as mybir
import concourse.tile as tile
from concourse import bacc
from concourse.bass_utils import run_bass_kernel_spmd

B, T, C = 4, 2048, 1024
H, D = 16, 64
HL = H // 2            # heads per core
NP = HL // 2           # head pairs per core
KC = C // 128          # contraction chunks for qkv projection
NT1 = T // 512         # 512-wide query blocks
NT2 = T // 128         # 128-tall key tiles
F32 = mybir.dt.float32
F16 = mybir.dt.float16
NPH = np.float16
EXP = mybir.ActivationFunctionType.Exp
IDN = mybir.ActivationFunctionType.Identity

_cached = {}


def install_profile_hook():
    """The agent image's antenv lacks axon_hooks; synthesize it so
    run_bass_kernel_spmd(trace=True) can capture NTFF profiles."""
    import types
    if 'antenv.axon_hooks' in sys.modules:
        return
    mod = types.ModuleType('antenv.axon_hooks')
    mod._hook = None

    def set_axon_ntff_profile_hook(h):
        mod._hook = h

    def get_axon_ntff_profile_hook():
        return mod._hook

    mod.set_axon_ntff_profile_hook = set_axon_ntff_profile_hook
    mod.get_axon_ntff_profile_hook = get_axon_ntff_profile_hook
    sys.modules['antenv.axon_hooks'] = mod
    try:
        from trn_agent_boot.trn_boot import _ntff_profile_via_ctypes
        set_axon_ntff_profile_hook(_ntff_profile_via_ctypes('/opt/axon/libaxon_pjrt.so'))
    except Exception as e:
        print(f"profile hook install failed: {e}", file=sys.stderr)


def build_kernel():
    if 'nc' in _cached:
        return _cached['nc']
    nc = bacc.Bacc("TRN2", target_bir_lowering=False, debug=False, num_devices=8)

    xT = nc.declare_dram_parameter("xT", [C, T], F16, isOutput=False)
    w_qk = nc.declare_dram_parameter("w_qk", [C, 2 * HL * D], F16, isOutput=False)
    w_v = nc.declare_dram_parameter("w_v", [C, HL * D], F16, isOutput=False)
    b_qk = nc.declare_dram_parameter("b_qk", [2 * HL * D, 1], F32, isOutput=False)
    b_v = nc.declare_dram_parameter("b_v", [1, HL * D], F16, isOutput=False)
    w_proj = nc.declare_dram_parameter("w_proj", [HL * D, C], F16, isOutput=False)
    b_proj_half = nc.declare_dram_parameter("b_proj_half", [1, C], F16, isOutput=False)
    y_out = nc.declare_dram_parameter("y", [T, C], F16, isOutput=True)

    with tile.TileContext(nc) as tc, ExitStack() as st:
        cpool = st.enter_context(tc.tile_pool(name="const", bufs=1))
        v_pool = st.enter_context(tc.tile_pool(name="vstore", bufs=1))
        qk_pool = st.enter_context(tc.tile_pool(name="qkT", bufs=1))
        xpool = st.enter_context(tc.tile_pool(name="xT", bufs=1))
        wpool = st.enter_context(tc.tile_pool(name="wqk", bufs=1))
        o_pool = st.enter_context(tc.tile_pool(name="outT", bufs=1, side="right"))

        # ---- constants ----
        ones128h = cpool.tile([1, 128], F16)
        nc.gpsimd.memset(ones128h[:], 1.0)
        ones_p = cpool.tile([128, HL], F16)
        nc.gpsimd.memset(ones_p[:], 1.0)
        bqk_sb = cpool.tile([128, 2 * NP, 1], F32)
        nc.gpsimd.dma_start(bqk_sb[:], b_qk[:].rearrange("(c p) o -> p c o", p=128))

        # persistent tiles
        vst = [v_pool.tile([128, HL, D + 1], F16, tag=f"vs{m}", name=f"vs{m}")
               for m in range(NT2)]
        outT = [o_pool.tile([128, T], F16, tag=f"o{j}", name=f"o{j}")
                for j in range(NP)]
        xTt = [xpool.tile([128, T], F16, tag=f"x{kc}", name=f"x{kc}")
               for kc in range(KC)]
        wqk_sb = [wpool.tile([128, 2 * HL * D], F16, tag=f"w{kc}", name=f"w{kc}")
                  for kc in range(KC)]
        wv_sb = [wpool.tile([128, HL * D], F16, tag=f"wv{kc}", name=f"wv{kc}")
                 for kc in range(KC)]
        wp_sb = [wpool.tile([128, C], F16, tag=f"wp{j}", name=f"wp{j}")
                 for j in range(NP)]
        qkT = [qk_pool.tile([128, T], F16, tag=f"qk{j}", name=f"qk{j}")
               for j in range(2 * NP)]

        # ---- input DMA: parallel issue across three queues; pass-0 x
        # columns first so the kc-outer accumulation starts asap ----
        # pass-0 operands arrive in consumption order: per kc, three 128KB
        # pieces land in parallel on the three queues (~1.5us/round), pacing
        # the 1.7us/kc accumulation with no cliff
        for kc in range(KC):
            nc.sync.dma_start(xTt[kc][:, 0:512], xT[bass.ts(kc, 128), 0:512])
            nc.scalar.dma_start(wqk_sb[kc][:, 0:512],
                                w_qk[bass.ts(kc, 128), 0:512])
            nc.gpsimd.dma_start(wqk_sb[kc][:, 512:1024],
                                w_qk[bass.ts(kc, 128), 512:1024])
        for kc in range(KC):
            nc.sync.dma_start(xTt[kc][:, 512:T], xT[bass.ts(kc, 128), 512:T])
            nc.gpsimd.dma_start(wv_sb[kc][:], w_v[bass.ts(kc, 128), :])
        bv_sb = wpool.tile([1, HL * D], F16, tag="bv_sb")
        nc.gpsimd.dma_start(bv_sb[:], b_v[:])
        for j in range(NP):
            nc.gpsimd.dma_start(wp_sb[j][:], w_proj[bass.ts(j, 128), :])
        bp_sb = wpool.tile([1, C], F16, tag="bp_sb")
        nc.gpsimd.dma_start(bp_sb[:], b_proj_half[:])

        # paired interleave: q-chunk j then k-chunk NP+j
        oc_order = [oc for j in range(NP) for oc in (j, NP + j)]

        # ---- stage A prefix: q/k pass for t[0:512], kc-outer over 8 PSUM
        # accumulators; drains (+bias) on the then-idle scalar engine ----
        with tc.tile_pool(name="aps", bufs=1, space="PSUM") as qps:
            acc = [qps.tile([128, 512], F32, tag=f"acc{oc}", bufs=1,
                            name=f"acc{oc}")
                   for oc in range(2 * NP)]
            for kc in range(KC):
                for oi, oc in enumerate(oc_order):
                    nc.tensor.matmul(
                        acc[oi][:], wqk_sb[kc][:, bass.ts(oc, 128)],
                        xTt[kc][:, 0:512],
                        start=(kc == 0), stop=(kc == KC - 1))
            for oi, oc in enumerate(oc_order):
                # alternate drain engines so neither serializes the handoff
                if oi % 2 == 0:
                    nc.scalar.activation(qkT[oc][:, 0:512], acc[oi][:], IDN,
                                         bias=bqk_sb[:, oc, :])
                else:
                    nc.vector.tensor_scalar_add(
                        qkT[oc][:, 0:512], acc[oi][:], bqk_sb[:, oc, :])

        # ---- main section: attention with fill jobs in the exp-gated PE
        # idle slots. All fill jobs share one 2-deep PSUM ring. ----
        with tc.tile_pool(name="ptile", bufs=4) as ppool, \
             tc.tile_pool(name="ytile", bufs=4) as ypool, \
             tc.tile_pool(name="misc", bufs=1, space="PSUM") as mps, \
             tc.tile_pool(name="s_ps", bufs=1, space="PSUM") as sps, \
             tc.tile_pool(name="pv_ps", bufs=1, space="PSUM") as pvps:

            bvb = wpool.tile([128, HL, D], F32, tag="bvb")
            bpb = ypool.tile([128, C], F32, tag="bpb", bufs=1)

            def emit_bvb():
                ps = mps.tile([128, 512], F32, tag="job", bufs=2, name="jps")
                nc.tensor.matmul(ps[:, 0:HL * D], ones128h[:], bv_sb[:],
                                 start=True, stop=True)
                nc.vector.tensor_copy(
                    bvb[:], ps[:, 0:HL * D].rearrange("p (h d) -> p h d", h=HL))

            def emit_bpb():
                # proj bias broadcast: first needed by blk1's proj jobs, so
                # keep it out of the pass-0 -> attention handoff window
                for n in range(2):
                    ps = mps.tile([128, 512], F32, tag="job", bufs=2, name="jps")
                    nc.tensor.matmul(ps[:], ones128h[:], bp_sb[:, bass.ts(n, 512)],
                                     start=True, stop=True)
                    nc.vector.tensor_copy(bpb[:, bass.ts(n, 512)], ps[:])

            def emit_v_job(m):
                ps = mps.tile([128, 512], F32, tag="job", bufs=2, name="jps")
                for kc in range(KC):
                    nc.tensor.matmul(
                        ps[:], xTt[kc][:, bass.ts(m, 128)], wv_sb[kc][:],
                        start=(kc == 0), stop=(kc == KC - 1))
                nc.vector.tensor_add(
                    vst[m][:, :, 0:D],
                    ps[:].rearrange("p (h d) -> p h d", h=HL), bvb[:])
                nc.vector.tensor_copy(vst[m][:, :, D], ones_p[:])

            def emit_qk_job(oc, n, drain_act=False):
                ps = mps.tile([128, 512], F32, tag="job", bufs=2, name="jps")
                for kc in range(KC):
                    nc.tensor.matmul(
                        ps[:], wqk_sb[kc][:, bass.ts(oc, 128)],
                        xTt[kc][:, bass.ts(n, 512)],
                        start=(kc == 0), stop=(kc == KC - 1))
                if drain_act:
                    # blk0 is DVE-bound (norm chains); route half the drains
                    # of the jobs consumed there to the then-lighter scalar
                    # engine
                    nc.scalar.activation(qkT[oc][:, bass.ts(n, 512)], ps[:],
                                         IDN, bias=bqk_sb[:, oc, :])
                else:
                    nc.vector.tensor_scalar_add(
                        qkT[oc][:, bass.ts(n, 512)], ps[:], bqk_sb[:, oc, :])

            def emit_proj_job(mt, n):
                ps = mps.tile([128, 512], F32, tag="job", bufs=2, name="jps")
                for j in range(NP):
                    nc.tensor.matmul(
                        ps[:], outT[j][:, bass.ts(mt, 128)],
                        wp_sb[j][:, bass.ts(n, 512)],
                        start=(j == 0), stop=(j == NP - 1))
                yt = ypool.tile([128, 512], F16, tag="yt", bufs=6)
                nc.vector.tensor_add(yt[:], ps[:], bpb[:, bass.ts(n, 512)])
                nc.sync.dma_start(
                    y_out[bass.ts(mt, 128), bass.ts(n, 512)], yt[:])

            emit_bvb()
            for m in range(4):
                emit_v_job(m)
            emit_bpb()

            # fill queues: per block, jobs that must complete before the NEXT
            # block starts (q/k chunks and v tiles), then projection jobs.
            fill = []          # deadline jobs, consumed at pair starts
            proj_pending = []  # no deadline until the end

            for blk in range(NT1):
                t1 = bass.ds(blk * 512, 512)
                nt2 = 4 * (blk + 1)
                if blk < NT1 - 1:
                    nxt = blk + 1
                    fill = ([(emit_qk_job, (oc, nxt, nxt == 1 and qi % 2 == 0))
                             for qi, oc in enumerate(oc_order)]
                            + [(emit_v_job, (m,)) for m in range(4 * nxt, 4 * nxt + 4)]
                            + fill)
                for j in range(NP):
                    # drip-feed deadline jobs, then projection jobs, into this
                    # pair's exp-gated PE idle; in late (longer) pairs half the
                    # budget is held back until mid-pair, where the exp
                    # backlog has accumulated
                    budget = 3 if blk == 0 else 3 + blk
                    start_budget = budget if blk < 2 else (budget + 1) // 2

                    def emit_fills(k):
                        for _ in range(k):
                            if fill:
                                fn, args = fill.pop(0)
                                fn(*args)
                            elif proj_pending:
                                emit_proj_job(*proj_pending.pop(0))

                    emit_fills(start_budget)
                    q_t, k_t = qkT[j], qkT[NP + j]
                    pv1 = pvps.tile([D + 1, 512], F32, tag="pvA", bufs=1)
                    pv2 = pvps.tile([D + 1, 512], F32, tag="pvB", bufs=1)
                    for i in range(nt2):
                        if i == nt2 // 2:
                            emit_fills(budget - start_budget)
                        t2 = bass.ds(i * 128, 128)
                        # diagonal tiles: only query columns >= lo are causally
                        # reachable from this key tile; skip the rest entirely
                        off = i * 128 - blk * 512
                        lo = max(off, 0)
                        tq = bass.ds(blk * 512 + lo, 512 - lo)
                        sAB = sps.tile([128, 2, 512], F32, tag="sAB", bufs=2)
                        nc.tensor.matmul(sAB[:, 0, lo:512], k_t[0:64, t2],
                                         q_t[0:64, tq],
                                         start=True, stop=True, tile_position=(0, 0))
                        nc.tensor.matmul(sAB[:, 1, lo:512], k_t[64:128, t2],
                                         q_t[64:128, tq],
                                         start=True, stop=True, tile_position=(64, 0))
                        pAB = ppool.tile([128, 2, 512], F16, tag="pAB", bufs=12)
                        nc.scalar.activation(
                            pAB[:, :, lo:512], sAB[:, :, lo:512], EXP,
                            scale=0.125)
                        if off >= 0:
                            # triangular strip: keep p[a, g, lo+ci] iff ci>=a
                            nc.gpsimd.affine_select(
                                out=pAB[:, :, lo:lo + 128],
                                in_=pAB[:, :, lo:lo + 128],
                                compare_op=mybir.AluOpType.is_ge, fill=0.0,
                                base=0, pattern=[[0, 2], [1, 128]],
                                channel_multiplier=-1)
                        nc.tensor.matmul(pv1[:, lo:512], vst[i][:, 2 * j, :],
                                         pAB[:, 0, lo:512],
                                         start=(i == 0), stop=(i == nt2 - 1))
                        nc.tensor.matmul(pv2[:, lo:512], vst[i][:, 2 * j + 1, :],
                                         pAB[:, 1, lo:512],
                                         start=(i == 0), stop=(i == nt2 - 1))
                    for h, pv in ((0, pv1), (1, pv2)):
                        rs_sb = ppool.tile([1, 512], F32, tag="rs_sb", bufs=4)
                        nc.vector.tensor_copy(rs_sb[:], pv[D:D + 1, :])
                        rec = ppool.tile([1, 512], F32, tag="rec", bufs=4)
                        nc.vector.reciprocal_approx_fast(rec[:], rs_sb[:])
                        rb = ppool.tile([64, 512], F32, tag="rb", bufs=4)
                        nc.gpsimd.partition_broadcast(rb[:], rec[:])
                        nc.vector.tensor_mul(outT[j][h * 64:(h + 1) * 64, t1],
                                             pv[0:D, :], rb[:])
                while fill:
                    fn, args = fill.pop(0)
                    fn(*args)
                if blk < NT1 - 1:
                    proj_pending.extend(
                        (mt, n) for mt in range(4 * blk, 4 * blk + 4)
                        for n in range(2))
            while proj_pending:
                emit_proj_job(*proj_pending.pop(0))

            # final block's projection: pre-run the pair-0..2 contractions of
            # four jobs at a time inside the last pair's normalization window
            # (accumulators borrowed from the now-idle sAB ring, j-interleaved
            # so no matmul head-of-line-blocks another); only the pair-3
            # matmuls and drains remain after the normalization completes.
            jobs = [(mt, n) for mt in range(4 * NT1 - 4, 4 * NT1)
                    for n in range(2)]
            for wave in (jobs[0:4], jobs[4:8]):
                accs = []
                for w in range(2):
                    sT = sps.tile([128, 2, 512], F32, tag="sAB", bufs=2,
                                  name="sT")
                    accs += [sT[:, 0, :], sT[:, 1, :]]
                for j in range(NP - 1):
                    for a, (mt, n) in zip(accs, wave):
                        nc.tensor.matmul(a, outT[j][:, bass.ts(mt, 128)],
                                         wp_sb[j][:, bass.ts(n, 512)],
                                         start=(j == 0), stop=False)
                for a, (mt, n) in zip(accs, wave):
                    nc.tensor.matmul(a, outT[NP - 1][:, bass.ts(mt, 128)],
                                     wp_sb[NP - 1][:, bass.ts(n, 512)],
                                     start=False, stop=True)
                    yt = ypool.tile([128, 512], F16, tag="yt", bufs=6)
                    nc.vector.tensor_add(yt[:], a, bpb[:, bass.ts(n, 512)])
                    nc.sync.dma_start(
                        y_out[bass.ts(mt, 128), bass.ts(n, 512)], yt[:])

    nc.compile()
    _cached['nc'] = nc
    return nc


def make_in_maps(x, w_qkv, b_qkv, w_proj, b_proj):
    x = np.asarray(x, dtype=np.float32)
    w_qkv = np.asarray(w_qkv, dtype=np.float32)
    b_qkv = np.asarray(b_qkv, dtype=np.float32)
    w_proj = np.asarray(w_proj, dtype=np.float32)
    b_proj = np.asarray(b_proj, dtype=np.float32)

    in_maps = []
    for c in range(8):
        b, g = c // 2, c % 2
        heads = list(range(g * HL, (g + 1) * HL))
        # paired column order: chunk j = [q(h_{2j}) | q(h_{2j+1})], then k chunks
        qcols, kcols = [], []
        for j in range(NP):
            for h in (heads[2 * j], heads[2 * j + 1]):
                qcols.extend(range(h * D, (h + 1) * D))
                kcols.extend(range(C + h * D, C + (h + 1) * D))
        vcols = [2 * C + h * D + d for h in heads for d in range(D)]
        qk_idx = np.array(qcols + kcols)
        v_idx = np.array(vcols)
        p_idx = np.array([h * D + d for h in heads for d in range(D)])

        in_maps.append({
            "xT": np.ascontiguousarray(x[b].T.astype(NPH)),
            "w_qk": np.ascontiguousarray(w_qkv[:, qk_idx].astype(NPH)),
            "w_v": np.ascontiguousarray(w_qkv[:, v_idx].astype(NPH)),
            "b_qk": np.ascontiguousarray(b_qkv[qk_idx][:, None]),
            "b_v": np.ascontiguousarray(b_qkv[v_idx][None, :].astype(NPH)),
            "w_proj": np.ascontiguousarray(w_proj[p_idx, :].astype(NPH)),
            "b_proj_half": np.ascontiguousarray(0.5 * b_proj[None, :].astype(NPH)),
        })
    return in_maps


def run(inputs, trace=False):
    if trace:
        install_profile_hook()
    nc = build_kernel()
    in_maps = make_in_maps(**inputs)
    res = run_bass_kernel_spmd(nc, in_maps, list(range(8)), trace=trace)
    out = np.empty((B, T, C), dtype=np.float32)
    for b in range(B):
        out[b] = res.results[2 * b]["y"].astype(np.float32) \
            + res.results[2 * b + 1]["y"].astype(np.float32)
    return out, res


def kernel(**inputs) -> np.ndarray:
    out, _ = run(inputs, trace=False)
    return out


# revision 38
# speedup vs baseline: 1.0068x; 1.0068x over previous
"""Causal multi-head attention block (B=4, T=2048, C=1024, H=16, D=64) on 8 trn2 cores.

Sharding: core c -> (batch b = c//2, head-group g = c%2 covering heads 8g..8g+8).
Each core computes qkv projection for its batch restricted to its 8 heads,
flash-style causal attention in transposed orientation, and a partial output
projection written to DRAM in fp16; the host sums the two head-group partials
per batch during unshard (no device collective).

The attention inner loop is exp-throughput-bound on the scalar engine, so the
qkv projection is software-pipelined into it: only the first 512-column q/k
pass (kc-outer over 8 PSUM accumulators, started as soon as the first x/w DMA
chunks land) and the first four v tiles run up front; the remaining q/k
chunks, v tiles, and all output-projection jobs are drip-fed into the
ACT-gated tensor-engine idle slots between attention pairs, all sharing one
2-deep PSUM "job" ring. Causal masking is a single gpsimd affine_select on
the probability tile; diagonal tiles only compute the causally-live query
columns.

No transposes are needed on-device: q/k are produced in (d, t) layout (which
both the score matmul and its row-packed pairing want), v in (t, d) layout
(what PV wants), and the attention output appears directly in the (c_local, t)
layout that the output projection consumes as its stationary operand. Softmax
denominators come from a ones-column appended to v (M=65 PV matmuls).
"""
import sys

sys.path.insert(0, '/opt/trn_rl_repo')

from contextlib import ExitStack

import numpy as np

import concourse.bass as bass
import concourse.mybir as mybir
import concourse.tile as tile
from concourse import bacc
from concourse.bass_utils import run_bass_kernel_spmd

B, T, C = 4, 2048, 1024
H, D = 16, 64
HL = H // 2            # heads per core
NP = HL // 2           # head pairs per core
KC = C // 128          # contraction chunks for qkv projection
NT1 = T // 512         # 512-wide query blocks
NT2 = T // 128         # 128-tall key tiles
F32 = mybir.dt.float32
F16 = mybir.dt.float16
NPH = np.float16
EXP = mybir.ActivationFunctionType.Exp
IDN = mybir.ActivationFunctionType.Identity

_cached = {}


def install_profile_hook():
    """The agent image's antenv lacks axon_hooks; synthesize it so
    run_bass_kernel_spmd(trace=True) can capture NTFF profiles."""
    import types
    if 'antenv.axon_hooks' in sys.modules:
        return
    mod = types.ModuleType('antenv.axon_hooks')
    mod._hook = None

    def set_axon_ntff_profile_hook(h):
        mod._hook = h

    def get_axon_ntff_profile_hook():
        return mod._hook

    mod.set_axon_ntff_profile_hook = set_axon_ntff_profile_hook
    mod.get_axon_ntff_profile_hook = get_axon_ntff_profile_hook
    sys.modules['antenv.axon_hooks'] = mod
    try:
        from trn_agent_boot.trn_boot import _ntff_profile_via_ctypes
        set_axon_ntff_profile_hook(_ntff_profile_via_ctypes('/opt/axon/libaxon_pjrt.so'))
    except Exception as e:
        print(f"profile hook install failed: {e}", file=sys.stderr)


def build_kernel():
    if 'nc' in _cached:
        return _cached['nc']
    nc = bacc.Bacc("TRN2", target_bir_lowering=False, debug=False, num_devices=8)

    xT = nc.declare_dram_parameter("xT", [C, T], F16, isOutput=False)
    w_qk = nc.declare_dram_parameter("w_qk", [C, 2 * HL * D], F16, isOutput=False)
    w_v = nc.declare_dram_parameter("w_v", [C, HL * D], F16, isOutput=False)
    b_qk = nc.declare_dram_parameter("b_qk", [2 * HL * D, 1], F32, isOutput=False)
    b_v = nc.declare_dram_parameter("b_v", [1, HL * D], F16, isOutput=False)
    w_proj = nc.declare_dram_parameter("w_proj", [HL * D, C], F16, isOutput=False)
    b_proj_half = nc.declare_dram_parameter("b_proj_half", [1, C], F16, isOutput=False)
    y_out = nc.declare_dram_parameter("y", [T, C], F16, isOutput=True)

    with tile.TileContext(nc) as tc, ExitStack() as st:
        cpool = st.enter_context(tc.tile_pool(name="const", bufs=1))
        v_pool = st.enter_context(tc.tile_pool(name="vstore", bufs=1))
        qk_pool = st.enter_context(tc.tile_pool(name="qkT", bufs=1))
        xpool = st.enter_context(tc.tile_pool(name="xT", bufs=1))
        wpool = st.enter_context(tc.tile_pool(name="wqk", bufs=1))
        o_pool = st.enter_context(tc.tile_pool(name="outT", bufs=1, side="right"))

        # ---- constants ----
        ones128h = cpool.tile([1, 128], F16)
        nc.gpsimd.memset(ones128h[:], 1.0)
        ones_p = cpool.tile([128, HL], F16)
        nc.gpsimd.memset(ones_p[:], 1.0)
        bqk_sb = cpool.tile([128, 2 * NP, 1], F32)
        nc.gpsimd.dma_start(bqk_sb[:], b_qk[:].rearrange("(c p) o -> p c o", p=128))

        # persistent tiles
        vst = [v_pool.tile([128, HL, D + 1], F16, tag=f"vs{m}", name=f"vs{m}")
               for m in range(NT2)]
        outT = [o_pool.tile([128, T], F16, tag=f"o{j}", name=f"o{j}")
                for j in range(NP)]
        xTt = [xpool.tile([128, T], F16, tag=f"x{kc}", name=f"x{kc}")
               for kc in range(KC)]
        wqk_sb = [wpool.tile([128, 2 * HL * D], F16, tag=f"w{kc}", name=f"w{kc}")
                  for kc in range(KC)]
        wv_sb = [wpool.tile([128, HL * D], F16, tag=f"wv{kc}", name=f"wv{kc}")
                 for kc in range(KC)]
        wp_sb = [wpool.tile([128, C], F16, tag=f"wp{j}", name=f"wp{j}")
                 for j in range(NP)]
        qkT = [qk_pool.tile([128, T], F16, tag=f"qk{j}", name=f"qk{j}")
               for j in range(2 * NP)]

        # ---- input DMA: parallel issue across three queues; pass-0 x
        # columns first so the kc-outer accumulation starts asap ----
        # pass-0 operands arrive in consumption order: per kc, three 128KB
        # pieces land in parallel on the three queues (~1.5us/round), pacing
        # the 1.7us/kc accumulation with no cliff
        for kc in range(KC):
            nc.sync.dma_start(xTt[kc][:, 0:512], xT[bass.ts(kc, 128), 0:512])
            nc.scalar.dma_start(wqk_sb[kc][:, 0:512],
                                w_qk[bass.ts(kc, 128), 0:512])
            nc.gpsimd.dma_start(wqk_sb[kc][:, 512:1024],
                                w_qk[bass.ts(kc, 128), 512:1024])
        for kc in range(KC):
            nc.sync.dma_start(xTt[kc][:, 512:T], xT[bass.ts(kc, 128), 512:T])
            nc.gpsimd.dma_start(wv_sb[kc][:], w_v[bass.ts(kc, 128), :])
        bv_sb = wpool.tile([1, HL * D], F16, tag="bv_sb")
        nc.gpsimd.dma_start(bv_sb[:], b_v[:])
        for j in range(NP):
            nc.gpsimd.dma_start(wp_sb[j][:], w_proj[bass.ts(j, 128), :])
        bp_sb = wpool.tile([1, C], F16, tag="bp_sb")
        nc.gpsimd.dma_start(bp_sb[:], b_proj_half[:])

        # paired interleave: q-chunk j then k-chunk NP+j
        oc_order = [oc for j in range(NP) for oc in (j, NP + j)]

        # ---- stage A prefix: q/k pass for t[0:512], kc-outer over 8 PSUM
        # accumulators; drains (+bias) on the then-idle scalar engine ----
        with tc.tile_pool(name="aps", bufs=1, space="PSUM") as qps:
            acc = [qps.tile([128, 512], F32, tag=f"acc{oc}", bufs=1,
                            name=f"acc{oc}")
                   for oc in range(2 * NP)]
            for kc in range(KC):
                for oi, oc in enumerate(oc_order):
                    nc.tensor.matmul(
                        acc[oi][:], wqk_sb[kc][:, bass.ts(oc, 128)],
                        xTt[kc][:, 0:512],
                        start=(kc == 0), stop=(kc == KC - 1))
            for oi, oc in enumerate(oc_order):
                # alternate drain engines so neither serializes the handoff
                if oi % 2 == 0:
                    nc.scalar.activation(qkT[oc][:, 0:512], acc[oi][:], IDN,
                                         bias=bqk_sb[:, oc, :])
                else:
                    nc.vector.tensor_scalar_add(
                        qkT[oc][:, 0:512], acc[oi][:], bqk_sb[:, oc, :])

        # ---- main section: attention with fill jobs in the exp-gated PE
        # idle slots. All fill jobs share one 2-deep PSUM ring. ----
        with tc.tile_pool(name="ptile", bufs=4) as ppool, \
             tc.tile_pool(name="ytile", bufs=4) as ypool, \
             tc.tile_pool(name="misc", bufs=1, space="PSUM") as mps, \
             tc.tile_pool(name="s_ps", bufs=1, space="PSUM") as sps, \
             tc.tile_pool(name="pv_ps", bufs=1, space="PSUM") as pvps:

            bvb = wpool.tile([128, HL, D], F32, tag="bvb")
            bpb = ypool.tile([128, C], F32, tag="bpb", bufs=1)

            def emit_bvb():
                ps = mps.tile([128, 512], F32, tag="job", bufs=2, name="jps")
                nc.tensor.matmul(ps[:, 0:HL * D], ones128h[:], bv_sb[:],
                                 start=True, stop=True)
                nc.vector.tensor_copy(
                    bvb[:], ps[:, 0:HL * D].rearrange("p (h d) -> p h d", h=HL))

            def emit_bpb():
                # proj bias broadcast: first needed by blk1's proj jobs, so
                # keep it out of the pass-0 -> attention handoff window
                for n in range(2):
                    ps = mps.tile([128, 512], F32, tag="job", bufs=2, name="jps")
                    nc.tensor.matmul(ps[:], ones128h[:], bp_sb[:, bass.ts(n, 512)],
                                     start=True, stop=True)
                    nc.vector.tensor_copy(bpb[:, bass.ts(n, 512)], ps[:])

            def emit_v_job(m):
                ps = mps.tile([128, 512], F32, tag="job", bufs=2, name="jps")
                for kc in range(KC):
                    nc.tensor.matmul(
                        ps[:], xTt[kc][:, bass.ts(m, 128)], wv_sb[kc][:],
                        start=(kc == 0), stop=(kc == KC - 1))
                nc.vector.tensor_add(
                    vst[m][:, :, 0:D],
                    ps[:].rearrange("p (h d) -> p h d", h=HL), bvb[:])
                nc.vector.tensor_copy(vst[m][:, :, D], ones_p[:])

            def emit_qk_job(oc, n, drain_act=False):
                ps = mps.tile([128, 512], F32, tag="job", bufs=2, name="jps")
                for kc in range(KC):
                    nc.tensor.matmul(
                        ps[:], wqk_sb[kc][:, bass.ts(oc, 128)],
                        xTt[kc][:, bass.ts(n, 512)],
                        start=(kc == 0), stop=(kc == KC - 1))
                if drain_act:
                    # blk0 is DVE-bound (norm chains); route half the drains
                    # of the jobs consumed there to the then-lighter scalar
                    # engine
                    nc.scalar.activation(qkT[oc][:, bass.ts(n, 512)], ps[:],
                                         IDN, bias=bqk_sb[:, oc, :])
                else:
                    nc.vector.tensor_scalar_add(
                        qkT[oc][:, bass.ts(n, 512)], ps[:], bqk_sb[:, oc, :])

            def emit_proj_job(mt, n):
                ps = mps.tile([128, 512], F32, tag="job", bufs=2, name="jps")
                for j in range(NP):
                    nc.tensor.matmul(
                        ps[:], outT[j][:, bass.ts(mt, 128)],
                        wp_sb[j][:, bass.ts(n, 512)],
                        start=(j == 0), stop=(j == NP - 1))
                yt = ypool.tile([128, 512], F16, tag="yt")
                nc.vector.tensor_add(yt[:], ps[:], bpb[:, bass.ts(n, 512)])
                nc.sync.dma_start(
                    y_out[bass.ts(mt, 128), bass.ts(n, 512)], yt[:])

            emit_bvb()
            for m in range(4):
                emit_v_job(m)
            emit_bpb()

            # fill queues: per block, jobs that must complete before the NEXT
            # block starts (q/k chunks and v tiles), then projection jobs.
            fill = []          # deadline jobs, consumed at pair starts
            proj_pending = []  # no deadline until the end

            for blk in range(NT1):
                t1 = bass.ds(blk * 512, 512)
                nt2 = 4 * (blk + 1)
                if blk < NT1 - 1:
                    nxt = blk + 1
                    fill = ([(emit_qk_job, (oc, nxt, nxt == 1 and qi % 2 == 0))
                             for qi, oc in enumerate(oc_order)]
                            + [(emit_v_job, (m,)) for m in range(4 * nxt, 4 * nxt + 4)]
                            + fill)
                for j in range(NP):
                    # drip-feed deadline jobs, then projection jobs, into this
                    # pair's exp-gated PE idle; in late (longer) pairs half the
                    # budget is held back until mid-pair, where the exp
                    # backlog has accumulated
                    budget = 3 if blk == 0 else 3 + blk
                    start_budget = budget if blk < 2 else (budget + 1) // 2

                    def emit_fills(k):
                        for _ in range(k):
                            if fill:
                                fn, args = fill.pop(0)
                                fn(*args)
                            elif proj_pending:
                                emit_proj_job(*proj_pending.pop(0))

                    emit_fills(start_budget)
                    q_t, k_t = qkT[j], qkT[NP + j]
                    pv1 = pvps.tile([D + 1, 512], F32, tag="pvA", bufs=1)
                    pv2 = pvps.tile([D + 1, 512], F32, tag="pvB", bufs=1)
                    for i in range(nt2):
                        if i == nt2 // 2:
                            emit_fills(budget - start_budget)
                        t2 = bass.ds(i * 128, 128)
                        # diagonal tiles: only query columns >= lo are causally
                        # reachable from this key tile; skip the rest entirely
                        off = i * 128 - blk * 512
                        lo = max(off, 0)
                        tq = bass.ds(blk * 512 + lo, 512 - lo)
                        sAB = sps.tile([128, 2, 512], F32, tag="sAB", bufs=2)
                        nc.tensor.matmul(sAB[:, 0, lo:512], k_t[0:64, t2],
                                         q_t[0:64, tq],
                                         start=True, stop=True, tile_position=(0, 0))
                        nc.tensor.matmul(sAB[:, 1, lo:512], k_t[64:128, t2],
                                         q_t[64:128, tq],
                                         start=True, stop=True, tile_position=(64, 0))
                        pAB = ppool.tile([128, 2, 512], F16, tag="pAB", bufs=10)
                        nc.scalar.activation(
                            pAB[:, :, lo:512], sAB[:, :, lo:512], EXP,
                            scale=0.125)
                        if off >= 0:
                            # triangular strip: keep p[a, g, lo+ci] iff ci>=a
                            nc.gpsimd.affine_select(
                                out=pAB[:, :, lo:lo + 128],
                                in_=pAB[:, :, lo:lo + 128],
                                compare_op=mybir.AluOpType.is_ge, fill=0.0,
                                base=0, pattern=[[0, 2], [1, 128]],
                                channel_multiplier=-1)
                        nc.tensor.matmul(pv1[:, lo:512], vst[i][:, 2 * j, :],
                                         pAB[:, 0, lo:512],
                                         start=(i == 0), stop=(i == nt2 - 1))
                        nc.tensor.matmul(pv2[:, lo:512], vst[i][:, 2 * j + 1, :],
                                         pAB[:, 1, lo:512],
                                         start=(i == 0), stop=(i == nt2 - 1))
                    for h, pv in ((0, pv1), (1, pv2)):
                        rs_sb = ppool.tile([1, 512], F32, tag="rs_sb", bufs=3)
                        nc.vector.tensor_copy(rs_sb[:], pv[D:D + 1, :])
                        rec = ppool.tile([1, 512], F32, tag="rec", bufs=3)
                        nc.vector.reciprocal_approx_fast(rec[:], rs_sb[:])
                        rb = ppool.tile([64, 512], F32, tag="rb", bufs=3)
                        nc.gpsimd.partition_broadcast(rb[:], rec[:])
                        nc.vector.tensor_mul(outT[j][h * 64:(h + 1) * 64, t1],
                                             pv[0:D, :], rb[:])
                while fill:
                    fn, args = fill.pop(0)
                    fn(*args)
                if blk < NT1 - 1:
                    proj_pending.extend(
                        (mt, n) for mt in range(4 * blk, 4 * blk + 4)
                        for n in range(2))
            while proj_pending:
                emit_proj_job(*proj_pending.pop(0))

            # final block's projection: pre-run the pair-0..2 contractions of
            # four jobs at a time inside the last pair's normalization window
            # (accumulators borrowed from the now-idle sAB ring, j-interleaved
            # so no matmul head-of-line-blocks another); only the pair-3
            # matmuls and drains remain after the normalization completes.
            jobs = [(mt, n) for mt in range(4 * NT1 - 4, 4 * NT1)
                    for n in range(2)]
            for wave in (jobs[0:4], jobs[4:8]):
                accs = []
                for w in range(2):
                    sT = sps.tile([128, 2, 512], F32, tag="sAB", bufs=2,
                                  name="sT")
                    accs += [sT[:, 0, :], sT[:, 1, :]]
                for j in range(NP - 1):
                    for a, (mt, n) in zip(accs, wave):
                        nc.tensor.matmul(a, outT[j][:, bass.ts(mt, 128)],
                                         wp_sb[j][:, bass.ts(n, 512)],
                                         start=(j == 0), stop=False)
                for a, (mt, n) in zip(accs, wave):
                    nc.tensor.matmul(a, outT[NP - 1][:, bass.ts(mt, 128)],
                                     wp_sb[NP - 1][:, bass.ts(n, 512)],
                                     start=False, stop=True)
                    yt = ypool.tile([128, 512], F16, tag="yt")
                    nc.vector.tensor_add(yt[:], a, bpb[:, bass.ts(n, 512)])
                    nc.sync.dma_start(
                        y_out[bass.ts(mt, 128), bass.ts(n, 512)], yt[:])

    nc.compile()
    _cached['nc'] = nc
    return nc


def make_in_maps(x, w_qkv, b_qkv, w_proj, b_proj):
    x = np.asarray(x, dtype=np.float32)
    w_qkv = np.asarray(w_qkv, dtype=np.float32)
    b_qkv = np.asarray(b_qkv, dtype=np.float32)
    w_proj = np.asarray(w_proj, dtype=np.float32)
    b_proj = np.asarray(b_proj, dtype=np.float32)

    in_maps = []
    for c in range(8):
        b, g = c // 2, c % 2
        heads = list(range(g * HL, (g + 1) * HL))
        # paired column order: chunk j = [q(h_{2j}) | q(h_{2j+1})], then k chunks
        qcols, kcols = [], []
        for j in range(NP):
            for h in (heads[2 * j], heads[2 * j + 1]):
                qcols.extend(range(h * D, (h + 1) * D))
                kcols.extend(range(C + h * D, C + (h + 1) * D))
        vcols = [2 * C + h * D + d for h in heads for d in range(D)]
        qk_idx = np.array(qcols + kcols)
        v_idx = np.array(vcols)
        p_idx = np.array([h * D + d for h in heads for d in range(D)])

        in_maps.append({
            "xT": np.ascontiguousarray(x[b].T.astype(NPH)),
            "w_qk": np.ascontiguousarray(w_qkv[:, qk_idx].astype(NPH)),
            "w_v": np.ascontiguousarray(w_qkv[:, v_idx].astype(NPH)),
            "b_qk": np.ascontiguousarray(b_qkv[qk_idx][:, None]),
            "b_v": np.ascontiguousarray(b_qkv[v_idx][None, :].astype(NPH)),
            "w_proj": np.ascontiguousarray(w_proj[p_idx, :].astype(NPH)),
            "b_proj_half": np.ascontiguousarray(0.5 * b_proj[None, :].astype(NPH)),
        })
    return in_maps


def run(inputs, trace=False):
    if trace:
        install_profile_hook()
    nc = build_kernel()
    in_maps = make_in_maps(**inputs)
    res = run_bass_kernel_spmd(nc, in_maps, list(range(8)), trace=trace)
    out = np.empty((B, T, C), dtype=np.float32)
    for b in range(B):
        out[b] = res.results[2 * b]["y"].astype(np.float32) \
            + res.results[2 * b + 1]["y"].astype(np.float32)
    return out, res


def kernel(**inputs) -> np.ndarray:
    out, _ = run(inputs, trace=False)
    return out
